# revision 7
# baseline (speedup 1.0000x reference)
"""DistSageConv on 8 TRN2 NeuronCores (Bass/Tile).

Reference computation:
    out  = x @ W1.T + b1                                  # [n_src, 128]
    out1 = segment_sum(out[src_ids], dst_ids, n_dst)      # [n_dst, 128]
    out5 = x[:n_dst] @ W2.T + b2
    return out5 + out1

Distribution (the module's own design): src nodes sharded across 8 cores;
each core projects its own src shard into a local bf16 table, gathers its
own-src edges' rows with SWDGE dma_gather, segment-reduces them with one-hot
matmuls on the PE (dst grouped into 128-row blocks), then the per-core
partial aggregates [40960, 128] are reduce-scattered to block owners with a
3-stage XOR recursive-halving exchange over SBUF remote DMA
(remote_dma_broadcast relative dests). Owners fuse x[:n_dst] @ W2.T (+bias,
+deg*b1 via an augmented K=258 matmul) and write their 5120-row output slab.

The per-core partial slab is stored in an XOR-permuted owner-major column
order (column j on core c holds block (((j//40) ^ c) * 40 + j % 40)), which
makes the send/recv/add slabs of every exchange stage the same static column
ranges on all cores: a single SPMD program with no data-dependent control.
"""
import sys
sys.path.insert(0, "/opt/trn_rl_repo")

import numpy as np
import ml_dtypes

import os
import concourse.bacc as bacc
import concourse.bass as bass
import concourse.mybir as mybir
import concourse.tile as tile
from concourse import library_config
from concourse.bass_utils import run_bass_kernel_spmd

# ---------------- problem constants (hardcoded per contract) --------------
P = 8                      # cores
N_SRC = 100000
N_DST = 40000
N_EDGES = 640000
INF = 256                  # in_feats
OUTF = 128                 # out_feats
SRC_SH = N_SRC // P        # 12500 src rows per core
SRC_PAD = 12800            # padded table rows (25 x 512)
NBLK = 320                 # padded dst blocks of 128 (40960 dst rows)
BPC = NBLK // P            # 40 blocks (columns) owned per core
DST_PAD = NBLK * 128       # 40960
CPC = 8192                 # gather chunk edges
TPC = CPC // 128           # 64 tiles per chunk

F32 = mybir.dt.float32
BF16 = mybir.dt.bfloat16
I16 = mybir.dt.int16

_CACHE = {}


# ============================ host-side prep ==============================

def _wrap_idxs(idx):
    """[n] int16 -> [128, n//16] wrapped in 16 partitions, replicated x8."""
    n = len(idx)
    w = np.zeros((128, n // 16), dtype=np.int16)
    for p in range(16):
        w[p, :] = idx[p::16]
    for r in range(1, 8):
        w[16 * r:16 * r + 16, :] = w[:16, :]
    return w


def _host_prep(x, W1, b1, W2, b2, src_ids, dst_ids):
    """Build per-core input arrays + the static tile->column schedule."""
    x = np.asarray(x, np.float32)
    W1 = np.asarray(W1, np.float32)
    W2 = np.asarray(W2, np.float32)
    b1 = np.asarray(b1, np.float32).reshape(-1)
    b2 = np.asarray(b2, np.float32).reshape(-1)
    src_ids = np.asarray(src_ids, np.int64)
    dst_ids = np.asarray(dst_ids, np.int64)

    owner = src_ids // SRC_SH                       # edge -> src-owner core
    blk = dst_ids // 128                            # edge -> dst block
    deg_full = np.bincount(dst_ids, minlength=DST_PAD).astype(np.float32)

    # per-(core, column) counts;  column j on core c holds block beta_c(j)
    per_core = []
    for c in range(P):
        m = owner == c
        e_src = (src_ids[m] - c * SRC_SH).astype(np.int64)
        e_dst = dst_ids[m]
        e_blk = blk[m]
        col = ((e_blk // BPC) ^ c) * BPC + (e_blk % BPC)
        order = np.argsort(col, kind="stable")
        per_core.append((e_src[order], e_dst[order], col[order]))

    counts = np.zeros((P, NBLK), dtype=np.int64)
    for c in range(P):
        counts[c] = np.bincount(per_core[c][2], minlength=NBLK)
    t_col = np.maximum(1, (counts.max(axis=0) + 127) // 128)   # tiles per col
    nt_tot = int(t_col.sum())
    nt_pad = ((nt_tot + TPC - 1) // TPC) * TPC
    nchunk = nt_pad // TPC
    col_base = np.zeros(NBLK + 1, dtype=np.int64)
    col_base[1:] = np.cumsum(t_col)

    # static schedule: per tile -> (col, start, stop) ;  col -1 => pad tile
    tile_col = np.full(nt_pad, -1, dtype=np.int64)
    for j in range(NBLK):
        tile_col[col_base[j]:col_base[j + 1]] = j

    in_maps = []
    iota = np.broadcast_to(np.arange(128, dtype=np.float32), (128, 128))
    iota = np.ascontiguousarray(iota.astype(ml_dtypes.bfloat16))
    W1T = np.ascontiguousarray(W1.T)                          # [256, 128]
    W2T_aug = np.concatenate([W2.T, b1[None, :], b2[None, :]], axis=0)
    W2T_aug = np.ascontiguousarray(W2T_aug.astype(np.float32))  # [258, 128]

    for c in range(P):
        e_src, e_dst, e_col = per_core[c]
        # scatter edges into padded per-column tile runs
        idx_arr = np.zeros(nt_pad * 128, dtype=np.int16)
        dst_arr = np.full(nt_pad * 128, -1.0, dtype=np.float32)
        cc = np.bincount(e_col, minlength=NBLK)
        # position of each edge inside its column run
        pos_in_col = np.zeros(len(e_col), dtype=np.int64)
        cstart = np.zeros(NBLK + 1, dtype=np.int64)
        cstart[1:] = np.cumsum(cc)
        pos_in_col = np.arange(len(e_col)) - cstart[e_col]
        gpos = col_base[e_col] * 128 + pos_in_col
        idx_arr[gpos] = e_src.astype(np.int16)
        e_blk_of_col = ((e_col // BPC) ^ c) * BPC + (e_col % BPC)
        dst_arr[gpos] = (e_dst - e_blk_of_col * 128).astype(np.float32)

        idx_dram = np.zeros((nchunk, 128, CPC // 16), dtype=np.int16)
        for ch in range(nchunk):
            idx_dram[ch] = _wrap_idxs(idx_arr[ch * CPC:(ch + 1) * CPC])
        # dstloc layout per chunk: [128 part(edge-in-tile), TPC]
        dst_dram = dst_arr.reshape(nchunk, TPC, 128).transpose(0, 2, 1)
        dst_dram = np.ascontiguousarray(dst_dram)

        xT = np.zeros((INF, SRC_PAD), dtype=np.float32)
        xT[:, :SRC_SH] = x[c * SRC_SH:(c + 1) * SRC_SH].T
        lo, hi = c * 5120, min((c + 1) * 5120, N_DST)
        xdT = np.zeros((INF + 2, 5120), dtype=np.float32)
        if hi > lo:
            xdT[:INF, :hi - lo] = x[lo:hi].T
        xdT[INF, :] = deg_full[c * 5120:(c + 1) * 5120]
        xdT[INF + 1, :] = 1.0

        in_maps.append({
            "xT": xT,
            "xdT": np.ascontiguousarray(xdT),
            "W1T": W1T,
            "W2Ta": W2T_aug,
            "iota": iota,
            "idx": idx_dram,
            "dstloc": dst_dram,
        })

    sched = {"t_col": t_col, "nchunk": nchunk, "tile_col": tile_col}
    return in_maps, sched


# ============================ device program ==============================

def _build(sched):
    t_col = sched["t_col"]
    nchunk = sched["nchunk"]
    tile_col = sched["tile_col"]
    nt_pad = nchunk * TPC
    COLW = 128 * 2            # bf16 bytes per column per partition (unused)

    nc = bacc.Bacc("TRN2", target_bir_lowering=False, debug=False,
                   num_devices=P)

    xT_d = nc.dram_tensor("xT", [INF, SRC_PAD], F32, kind="ExternalInput")
    xdT_d = nc.dram_tensor("xdT", [INF + 2, 5120], F32, kind="ExternalInput")
    W1T_d = nc.dram_tensor("W1T", [INF, OUTF], F32, kind="ExternalInput")
    W2Ta_d = nc.dram_tensor("W2Ta", [INF + 2, OUTF], F32, kind="ExternalInput")
    iota_d = nc.dram_tensor("iota", [128, 128], BF16, kind="ExternalInput")
    idx_d = nc.dram_tensor("idx", [nchunk, 128, CPC // 16], I16, kind="ExternalInput")
    dst_d = nc.dram_tensor("dstloc", [nchunk, 128, TPC], F32, kind="ExternalInput")
    out_d = nc.dram_tensor("out", [5120, OUTF], F32, kind="ExternalOutput")
    tab_d = nc.dram_tensor("tab", [SRC_PAD, OUTF], BF16, kind="Internal")

    # persistent SBUF: partial slab + exchange recv
    PART = nc.alloc_sbuf_tensor("part", [128, NBLK * 128], BF16)
    RECV = nc.alloc_sbuf_tensor("recv", [128, (NBLK // 2) * 128], BF16)

    rsem = nc.semaphore("rsem").__enter__()      # data arrived (remote inc)
    nsem = nc.semaphore("nsem").__enter__()      # notify: peer consumed recv
    lsem = nc.semaphore("lsem").__enter__()      # local send drained
    psem = nc.semaphore("psem").__enter__()      # desc-gen complete
    xsem = nc.semaphore("xsem").__enter__()      # DVE add done

    with tile.TileContext(nc) as tc:
        nc.gpsimd.load_library(library_config.mlp)
        with (
            tc.tile_pool(name="consts", bufs=1) as constp,
            tc.tile_pool(name="xab", bufs=3) as xabp,
            tc.tile_pool(name="xc2", bufs=2) as xc2p,
            tc.tile_pool(name="stage", bufs=3) as stagep,
            tc.tile_pool(name="idx", bufs=2) as idxp,
            tc.tile_pool(name="dstl", bufs=2) as dstlp,
            tc.tile_pool(name="gath", bufs=2) as gathp,
            tc.tile_pool(name="oh", bufs=6) as ohp,
            tc.tile_pool(name="ps", bufs=2, space="PSUM") as psp,
            tc.tile_pool(name="pssc", bufs=1, space="PSUM") as pssc,
        ):
            # ---- constants
            iota_t = constp.tile([128, 128], BF16)
            nc.sync.dma_start(iota_t[:], iota_d[:])
            w1 = constp.tile([128, 2, OUTF], F32)
            nc.sync.dma_start(w1[:], W1T_d[:].rearrange("(k p) f -> p k f", p=128))
            w2 = constp.tile([128, 2, OUTF], F32)
            nc.sync.dma_start(w2[:], W2Ta_d[:INF].rearrange("(k p) f -> p k f", p=128))
            wb = constp.tile([2, OUTF], F32)
            nc.sync.dma_start(wb[:], W2Ta_d[INF:INF + 2, :])

            # ---------------- phase 1: project own src shard ----------------
            with nc.named_scope("phase1"):
                for j in range(SRC_PAD // 512):
                    a0 = xabp.tile([128, 512], F32, tag="a0")
                    a1 = xabp.tile([128, 512], F32, tag="a1")
                    nc.sync.dma_start(a0[:], xT_d[0:128, j * 512:(j + 1) * 512])
                    nc.sync.dma_start(a1[:], xT_d[128:256, j * 512:(j + 1) * 512])
                    ps = psp.tile([128, 512], F32, space="PSUM", tag="ps")
                    for u in range(4):
                        nc.tensor.matmul(
                            out=ps[:, u * 128:(u + 1) * 128],
                            lhsT=a0[:, u * 128:(u + 1) * 128], rhs=w1[:, 0, :],
                            start=(u == 0), stop=False)
                        nc.tensor.matmul(
                            out=ps[:, u * 128:(u + 1) * 128],
                            lhsT=a1[:, u * 128:(u + 1) * 128], rhs=w1[:, 1, :],
                            start=False, stop=(u == 3))
                    st = stagep.tile([128, 512], BF16, tag="st1")
                    nc.vector.tensor_copy(out=st[:], in_=ps[:])
                    nc.sync.dma_start(
                        tab_d[j * 512:(j + 1) * 512, :].rearrange(
                            "(u p) f -> p u f", p=128),
                        st[:].rearrange("p (u f) -> p u f", u=4))

            # ---------------- phase 2: gather + segment matmul --------------
            if os.environ.get("SKIP_P2"):
                pass
            elif True:
             with nc.named_scope("phase2"):
                ps_g = None
                for ch in range(nchunk):
                    idx_t = idxp.tile([128, CPC // 16], I16)
                    nc.sync.dma_start(idx_t[:], idx_d[ch])
                    dst_t = dstlp.tile([128, TPC], F32)
                    nc.sync.dma_start(dst_t[:], dst_d[ch])
                    gt = gathp.tile([128, TPC, 128], BF16)
                    nc.gpsimd.dma_gather(gt[:], tab_d[:], idx_t[:], CPC, CPC, OUTF,
                                         single_packet=False)
                    for t in range(TPC):
                        g = ch * TPC + t
                        col = int(tile_col[g])
                        oh = ohp.tile([128, 128], BF16)
                        nc.vector.tensor_scalar(
                            out=oh[:], in0=iota_t[:], scalar1=dst_t[:, t:t + 1],
                            scalar2=None, op0=mybir.AluOpType.is_equal)
                        if col < 0:
                            sc = pssc.tile([128, 128], F32, space="PSUM", tag="sc")
                            nc.tensor.matmul(out=sc[:], lhsT=oh[:], rhs=gt[:, t, :],
                                             start=True, stop=True)
                            continue
                        u_in_col = g - int(np.sum(t_col[:col]))
                        grp, ucol = col // 4, col % 4
                        first = (ucol == 0 and u_in_col == 0)
                        last = (ucol == 3 and u_in_col == int(t_col[col]) - 1)
                        if first:
                            ps_g = psp.tile([128, 512], F32, space="PSUM", tag="ps")
                        nc.tensor.matmul(
                            out=ps_g[:, ucol * 128:(ucol + 1) * 128],
                            lhsT=oh[:], rhs=gt[:, t, :],
                            start=first, stop=last)
                        if last:
                            nc.vector.tensor_copy(
                                out=PART[:, grp * 512:(grp + 1) * 512],
                                in_=ps_g[:])

            # ---------------- exchange: XOR recursive-halving RS ------------
            tc.strict_bb_all_engine_barrier()
            if os.environ.get("SKIP_EXCHANGE"):
                pass
            elif True:
              with tc.tile_critical():
                with nc.named_scope("exchange"):
                    nc.gpsimd.load_library(library_config.remote_dma)
                    nc.gpsimd.bir_kernel_barrier_wait([list(range(P))])
                    stages = [
                        # (xor_k, send_col_lo, ncols, slots, decl)
                        (4, 160, 160, [4, 5, 6, 7], 6),
                        (2, 80, 80, [0, 1, 2, 3], 2),
                        (1, 40, 40, [0, 1, 2, 3], 1),
                    ]
                    npr = 0      # preps issued
                    for s, (k, lo, ncols, slots, decl) in enumerate(stages):
                        sub = ncols // 4
                        for i, slot in enumerate(slots):
                            rdests = [None] * 8
                            rdests[slot] = (0, decl)
                            nc.gpsimd.remote_dma_broadcast(
                                out_ap=RECV[:, (lo - ncols + i * sub) * 128:
                                            (lo - ncols + (i + 1) * sub) * 128],
                                in_ap=PART[:, (lo + i * sub) * 128:
                                           (lo + (i + 1) * sub) * 128],
                                remote_sem=rsem, local_sem=lsem,
                                rdests=rdests).then_inc(psem, 1)
                        npr += 4
                        nc.gpsimd.wait_ge(psem, npr)
                        nc.gpsimd.trigger_dma(count=4)
                        # wait for peer data, then reduce into kept half
                        nc.vector.wait_ge(rsem, 8 * (s + 1))
                        keep = lo - ncols
                        nc.vector.tensor_tensor(
                            out=PART[:, keep * 128:lo * 128],
                            in0=PART[:, keep * 128:lo * 128],
                            in1=RECV[:, keep * 128:lo * 128],
                            op=mybir.AluOpType.add).then_inc(xsem, 1)
                        if s < 2:
                            # notify next-stage peer that our recv buf is free
                            k2 = stages[s + 1][0]
                            nc.gpsimd.wait_ge(xsem, s + 1)
                            rdests = [None] * 8
                            rdests[0] = (0, k2)
                            nc.gpsimd.remote_sem_update_broadcast(
                                remote_sem=nsem, local_sem=lsem,
                                rdests=rdests).then_inc(psem, 1)
                            npr += 1
                            nc.gpsimd.wait_ge(psem, npr)
                            nc.gpsimd.trigger_dma(count=1)
                            nc.gpsimd.wait_ge(nsem, 2 * (s + 1))
                    nc.gpsimd.wait_ge(xsem, 3)
                    nc.gpsimd.wait_ge(lsem, 16 * npr)

            # ---------------- phase 3: own-dst projection + bias + add ------
            if os.environ.get("SKIP_P3"):
                ost0 = stagep.tile([128, 512], F32, tag="ost")
                nc.vector.tensor_copy(out=ost0[:], in_=PART[:, 0:512])
                nc.sync.dma_start(out_d[0:512, :].rearrange("(u p) f -> p u f", p=128),
                                  ost0[:].rearrange("p (u f) -> p u f", u=4))
            elif True:
             with nc.named_scope("phase3"):
                for grp in range(10):
                    b0 = xc2p.tile([128, 512], F32, tag="b0")
                    b1t = xc2p.tile([128, 512], F32, tag="b1")
                    b2t = xc2p.tile([2, 512], F32, tag="b2")
                    nc.sync.dma_start(b0[:], xdT_d[0:128, grp * 512:(grp + 1) * 512])
                    nc.sync.dma_start(b1t[:], xdT_d[128:256, grp * 512:(grp + 1) * 512])
                    nc.sync.dma_start(b2t[:], xdT_d[256:258, grp * 512:(grp + 1) * 512])
                    ps3 = psp.tile([128, 512], F32, space="PSUM", tag="ps")
                    for u in range(4):
                        sl = slice(u * 128, (u + 1) * 128)
                        nc.tensor.matmul(out=ps3[:, sl], lhsT=b0[:, sl],
                                         rhs=w2[:, 0, :], start=(u == 0), stop=False)
                        nc.tensor.matmul(out=ps3[:, sl], lhsT=b1t[:, sl],
                                         rhs=w2[:, 1, :], start=False, stop=False)
                        nc.tensor.matmul(out=ps3[:, sl], lhsT=b2t[:, sl],
                                         rhs=wb[:], start=False, stop=(u == 3))
                    ost = stagep.tile([128, 512], F32, tag="ost")
                    nc.vector.tensor_tensor(
                        out=ost[:], in0=ps3[:],
                        in1=PART[:, grp * 512:(grp + 1) * 512],
                        op=mybir.AluOpType.add)
                    nc.sync.dma_start(
                        out_d[grp * 512:(grp + 1) * 512, :].rearrange(
                            "(u p) f -> p u f", p=128),
                        ost[:].rearrange("p (u f) -> p u f", u=4))

    nc.compile()
    return nc


# ============================ public entry ================================

def _install_ntff_hook():
    """The agent image lacks antenv.axon_hooks; recreate it and register the
    ctypes NTFF profile hook so trace=True works under axon."""
    import types
    import antenv
    if "antenv.axon_hooks" not in sys.modules:
        m = types.ModuleType("antenv.axon_hooks")
        _h = [None]
        m.get_axon_ntff_profile_hook = lambda: _h[0]
        m.set_axon_ntff_profile_hook = lambda h: _h.__setitem__(0, h)
        sys.modules["antenv.axon_hooks"] = m
        antenv.axon_hooks = m
    import antenv.axon_hooks as ah
    if ah.get_axon_ntff_profile_hook() is None:
        try:
            from trn_agent_boot.trn_boot import _ntff_profile_via_ctypes
            ah.set_axon_ntff_profile_hook(
                _ntff_profile_via_ctypes("/opt/axon/libaxon_pjrt.so"))
        except Exception as e:
            print(f"ntff hook install failed ({e}); timing disabled")



def kernel(x, W1, b1, W2, b2, src_ids, dst_ids, n_dst):
    n_dst = int(n_dst)
    assert n_dst == N_DST
    in_maps, sched = _host_prep(x, W1, b1, W2, b2, src_ids, dst_ids)
    key = (sched["nchunk"], tuple(sched["t_col"].tolist()))
    if key not in _CACHE:
        _CACHE.clear()
        _CACHE[key] = _build(sched)
    nc = _CACHE[key]
    trace = bool(os.environ.get("BASS_KERNEL_TRACE"))
    kw = {}
    if trace:
        _install_ntff_hook()
        kw = dict(trace=True, trace_cores=[0], stitch_traces=False)
    res = run_bass_kernel_spmd(nc, in_maps, core_ids=list(range(P)), **kw)
    if trace:
        print(f"HW exec time: {res.exec_time_ns} ns")
        if res.per_core_scope_times:
            for scope, m in sorted(res.per_core_scope_times.items()):
                print(f"  scope {scope}: {m}")
        if res.instructions_and_trace:
            print(f"  trace: {res.instructions_and_trace[1]}")
    out = np.concatenate([res.results[c]["out"] for c in range(P)], axis=0)
    return np.ascontiguousarray(out[:N_DST]).astype(np.float32)


if __name__ == "__main__":
    # smoke test with random data
    rng = np.random.default_rng(0)
    x = rng.standard_normal((N_SRC, INF), dtype=np.float32)
    W1 = rng.standard_normal((OUTF, INF), dtype=np.float32) * 0.0625
    W2 = rng.standard_normal((OUTF, INF), dtype=np.float32) * 0.0625
    b1 = np.zeros(OUTF, np.float32)
    b2 = np.zeros(OUTF, np.float32)
    src = rng.integers(0, N_SRC, N_EDGES).astype(np.int32)
    dst = np.sort(rng.integers(0, N_DST, N_EDGES).astype(np.int32))
    got = kernel(x, W1, b1, W2, b2, src, dst, N_DST)
    proj = x @ W1.T + b1
    want = np.zeros((N_DST, OUTF), np.float32)
    np.add.at(want, dst, proj[src])
    want += x[:N_DST] @ W2.T + b2
    denom = np.abs(want).max()
    print("rel err:", np.abs(got - want).max() / denom)


# revision 8
# speedup vs baseline: 1.2087x; 1.2087x over previous
"""DistSageConv on 8 TRN2 NeuronCores (Bass/Tile).

Reference computation:
    out  = x @ W1.T + b1                                  # [n_src, 128]
    out1 = segment_sum(out[src_ids], dst_ids, n_dst)      # [n_dst, 128]
    out5 = x[:n_dst] @ W2.T + b2
    return out5 + out1

Distribution (the module's own design): src nodes sharded across 8 cores;
each core projects its own src shard into a local bf16 table, gathers its
own-src edges' rows with SWDGE dma_gather, segment-reduces them with one-hot
matmuls on the PE (dst grouped into 128-row blocks), then the per-core
partial aggregates [40960, 128] are reduce-scattered to block owners with a
3-stage XOR recursive-halving exchange over SBUF remote DMA
(remote_dma_broadcast relative dests). Owners fuse x[:n_dst] @ W2.T (+bias,
+deg*b1 via an augmented K=258 matmul) and write their 5120-row output slab.

The per-core partial slab is stored in an XOR-permuted owner-major column
order (column j on core c holds block (((j//40) ^ c) * 40 + j % 40)), which
makes the send/recv/add slabs of every exchange stage the same static column
ranges on all cores: a single SPMD program with no data-dependent control.
"""
import sys
sys.path.insert(0, "/opt/trn_rl_repo")

import numpy as np
import ml_dtypes

import os
import concourse.bacc as bacc
import concourse.bass as bass
import concourse.mybir as mybir
import concourse.tile as tile
from concourse import library_config
from concourse.bass_utils import run_bass_kernel_spmd

# ---------------- problem constants (hardcoded per contract) --------------
P = 8                      # cores
N_SRC = 100000
N_DST = 40000
N_EDGES = 640000
INF = 256                  # in_feats
OUTF = 128                 # out_feats
SRC_SH = N_SRC // P        # 12500 src rows per core
SRC_PAD = 12800            # padded table rows (25 x 512)
NBLK = 320                 # padded dst blocks of 128 (40960 dst rows)
BPC = NBLK // P            # 40 blocks (columns) owned per core
DST_PAD = NBLK * 128       # 40960
CPC = 4096                 # gather chunk edges
TPC = CPC // 128           # 64 tiles per chunk

F32 = mybir.dt.float32
BF16 = mybir.dt.bfloat16
I16 = mybir.dt.int16

_CACHE = {}


# ============================ host-side prep ==============================

def _wrap_idxs(idx):
    """[n] int16 -> [128, n//16] wrapped in 16 partitions, replicated x8."""
    n = len(idx)
    w = np.zeros((128, n // 16), dtype=np.int16)
    for p in range(16):
        w[p, :] = idx[p::16]
    for r in range(1, 8):
        w[16 * r:16 * r + 16, :] = w[:16, :]
    return w


def _host_prep(x, W1, b1, W2, b2, src_ids, dst_ids):
    """Build per-core input arrays + the static tile->column schedule."""
    x = np.asarray(x, np.float32)
    W1 = np.asarray(W1, np.float32)
    W2 = np.asarray(W2, np.float32)
    b1 = np.asarray(b1, np.float32).reshape(-1)
    b2 = np.asarray(b2, np.float32).reshape(-1)
    src_ids = np.asarray(src_ids, np.int64)
    dst_ids = np.asarray(dst_ids, np.int64)

    owner = src_ids // SRC_SH                       # edge -> src-owner core
    blk = dst_ids // 128                            # edge -> dst block
    deg_full = np.bincount(dst_ids, minlength=DST_PAD).astype(np.float32)

    # per-(core, column) counts;  column j on core c holds block beta_c(j)
    per_core = []
    for c in range(P):
        m = owner == c
        e_src = (src_ids[m] - c * SRC_SH).astype(np.int64)
        e_dst = dst_ids[m]
        e_blk = blk[m]
        col = ((e_blk // BPC) ^ c) * BPC + (e_blk % BPC)
        order = np.argsort(col, kind="stable")
        per_core.append((e_src[order], e_dst[order], col[order]))

    counts = np.zeros((P, NBLK), dtype=np.int64)
    for c in range(P):
        counts[c] = np.bincount(per_core[c][2], minlength=NBLK)
    t_col = np.maximum(1, (counts.max(axis=0) + 127) // 128)   # tiles per col
    nt_tot = int(t_col.sum())
    nt_pad = ((nt_tot + TPC - 1) // TPC) * TPC
    nchunk = nt_pad // TPC
    col_base = np.zeros(NBLK + 1, dtype=np.int64)
    col_base[1:] = np.cumsum(t_col)

    # static schedule: per tile -> (col, start, stop) ;  col -1 => pad tile
    tile_col = np.full(nt_pad, -1, dtype=np.int64)
    for j in range(NBLK):
        tile_col[col_base[j]:col_base[j + 1]] = j

    in_maps = []
    iota = np.broadcast_to(np.arange(128, dtype=np.float32), (128, 128))
    iota = np.ascontiguousarray(iota.astype(ml_dtypes.bfloat16))
    W1T = np.ascontiguousarray(W1.T)                          # [256, 128]
    W2T_aug = np.concatenate([W2.T, b1[None, :], b2[None, :]], axis=0)
    W2T_aug = np.ascontiguousarray(W2T_aug.astype(np.float32))  # [258, 128]

    for c in range(P):
        e_src, e_dst, e_col = per_core[c]
        # scatter edges into padded per-column tile runs
        idx_arr = np.zeros(nt_pad * 128, dtype=np.int16)
        dst_arr = np.full(nt_pad * 128, -1.0, dtype=np.float32)
        cc = np.bincount(e_col, minlength=NBLK)
        # position of each edge inside its column run
        pos_in_col = np.zeros(len(e_col), dtype=np.int64)
        cstart = np.zeros(NBLK + 1, dtype=np.int64)
        cstart[1:] = np.cumsum(cc)
        pos_in_col = np.arange(len(e_col)) - cstart[e_col]
        gpos = col_base[e_col] * 128 + pos_in_col
        idx_arr[gpos] = e_src.astype(np.int16)
        e_blk_of_col = ((e_col // BPC) ^ c) * BPC + (e_col % BPC)
        dst_arr[gpos] = (e_dst - e_blk_of_col * 128).astype(np.float32)

        idx_dram = np.zeros((nchunk, 128, CPC // 16), dtype=np.int16)
        for ch in range(nchunk):
            idx_dram[ch] = _wrap_idxs(idx_arr[ch * CPC:(ch + 1) * CPC])
        # dstloc layout per chunk: [128 part(edge-in-tile), TPC]
        dst_dram = dst_arr.reshape(nchunk, TPC, 128).transpose(0, 2, 1)
        dst_dram = np.ascontiguousarray(dst_dram.astype(ml_dtypes.bfloat16))

        xT = np.zeros((INF, SRC_PAD), dtype=np.float32)
        xT[:, :SRC_SH] = x[c * SRC_SH:(c + 1) * SRC_SH].T
        lo, hi = c * 5120, min((c + 1) * 5120, N_DST)
        xdT = np.zeros((INF + 2, 5120), dtype=np.float32)
        if hi > lo:
            xdT[:INF, :hi - lo] = x[lo:hi].T
        xdT[INF, :] = deg_full[c * 5120:(c + 1) * 5120]
        xdT[INF + 1, :] = 1.0

        in_maps.append({
            "xT": xT,
            "xdT": np.ascontiguousarray(xdT),
            "W1T": W1T,
            "W2Ta": W2T_aug,
            "iota": iota,
            "idx": idx_dram,
            "dstloc": dst_dram,
        })

    sched = {"t_col": t_col, "nchunk": nchunk, "tile_col": tile_col}
    return in_maps, sched


# ============================ device program ==============================

def _build(sched):
    t_col = sched["t_col"]
    nchunk = sched["nchunk"]
    tile_col = sched["tile_col"]
    nt_pad = nchunk * TPC
    COLW = 128 * 2            # bf16 bytes per column per partition (unused)

    nc = bacc.Bacc("TRN2", target_bir_lowering=False, debug=False,
                   num_devices=P)

    xT_d = nc.dram_tensor("xT", [INF, SRC_PAD], F32, kind="ExternalInput")
    xdT_d = nc.dram_tensor("xdT", [INF + 2, 5120], F32, kind="ExternalInput")
    W1T_d = nc.dram_tensor("W1T", [INF, OUTF], F32, kind="ExternalInput")
    W2Ta_d = nc.dram_tensor("W2Ta", [INF + 2, OUTF], F32, kind="ExternalInput")
    iota_d = nc.dram_tensor("iota", [128, 128], BF16, kind="ExternalInput")
    idx_d = nc.dram_tensor("idx", [nchunk, 128, CPC // 16], I16, kind="ExternalInput")
    dst_d = nc.dram_tensor("dstloc", [nchunk, 128, TPC], BF16, kind="ExternalInput")
    out_d = nc.dram_tensor("out", [5120, OUTF], F32, kind="ExternalOutput")
    tab_d = nc.dram_tensor("tab", [SRC_PAD, OUTF], BF16, kind="Internal")

    # persistent SBUF: partial slab + exchange recv
    PART = nc.alloc_sbuf_tensor("part", [128, NBLK * 128], BF16)
    RECV = nc.alloc_sbuf_tensor("recv", [128, (NBLK // 2) * 128], BF16)

    rsem = nc.semaphore("rsem").__enter__()      # data arrived (remote inc)
    nsem = nc.semaphore("nsem").__enter__()      # notify: peer consumed recv
    lsem = nc.semaphore("lsem").__enter__()      # local send drained
    psem = nc.semaphore("psem").__enter__()      # desc-gen complete
    xsem = nc.semaphore("xsem").__enter__()      # DVE add done

    with tile.TileContext(nc) as tc:
        nc.gpsimd.load_library(library_config.mlp)
        with (
            tc.tile_pool(name="consts", bufs=1) as constp,
            tc.tile_pool(name="xab", bufs=3) as xabp,
            tc.tile_pool(name="xc2", bufs=2) as xc2p,
            tc.tile_pool(name="stage", bufs=3) as stagep,
            tc.tile_pool(name="idx", bufs=2) as idxp,
            tc.tile_pool(name="dstl", bufs=2) as dstlp,
            tc.tile_pool(name="gath", bufs=2) as gathp,
            tc.tile_pool(name="oh", bufs=2) as ohp,
            tc.tile_pool(name="ps", bufs=2, space="PSUM") as psp,
            tc.tile_pool(name="pssc", bufs=1, space="PSUM") as pssc,
        ):
            # ---- constants
            iota_t = constp.tile([128, 128], BF16)
            nc.sync.dma_start(iota_t[:], iota_d[:])
            w1 = constp.tile([128, 2, OUTF], F32)
            nc.sync.dma_start(w1[:], W1T_d[:].rearrange("(k p) f -> p k f", p=128))
            w2 = constp.tile([128, 2, OUTF], F32)
            nc.sync.dma_start(w2[:], W2Ta_d[:INF].rearrange("(k p) f -> p k f", p=128))
            wb = constp.tile([2, OUTF], F32)
            nc.sync.dma_start(wb[:], W2Ta_d[INF:INF + 2, :])

            # ---------------- phase 1: project own src shard ----------------
            with nc.named_scope("phase1"):
                for j in range(SRC_PAD // 512):
                    a0 = xabp.tile([128, 512], F32, tag="a0")
                    a1 = xabp.tile([128, 512], F32, tag="a1")
                    nc.sync.dma_start(a0[:], xT_d[0:128, j * 512:(j + 1) * 512])
                    nc.sync.dma_start(a1[:], xT_d[128:256, j * 512:(j + 1) * 512])
                    ps = psp.tile([128, 512], F32, space="PSUM", tag="ps")
                    for u in range(4):
                        nc.tensor.matmul(
                            out=ps[:, u * 128:(u + 1) * 128],
                            lhsT=a0[:, u * 128:(u + 1) * 128], rhs=w1[:, 0, :],
                            start=(u == 0), stop=False)
                        nc.tensor.matmul(
                            out=ps[:, u * 128:(u + 1) * 128],
                            lhsT=a1[:, u * 128:(u + 1) * 128], rhs=w1[:, 1, :],
                            start=False, stop=(u == 3))
                    st = stagep.tile([128, 512], BF16, tag="st1")
                    nc.vector.tensor_copy(out=st[:], in_=ps[:])
                    nc.sync.dma_start(
                        tab_d[j * 512:(j + 1) * 512, :].rearrange(
                            "(u p) f -> p u f", p=128),
                        st[:].rearrange("p (u f) -> p u f", u=4))

            # ---------------- phase 2: gather + segment matmul --------------
            if os.environ.get("SKIP_P2"):
                pass
            elif True:
             with nc.named_scope("phase2"):
                ps_g = None
                for ch in range(nchunk):
                    idx_t = idxp.tile([128, CPC // 16], I16)
                    nc.sync.dma_start(idx_t[:], idx_d[ch])
                    dst_t = dstlp.tile([128, TPC], BF16)
                    nc.sync.dma_start(dst_t[:], dst_d[ch])
                    gt = gathp.tile([128, TPC, 128], BF16)
                    nc.gpsimd.dma_gather(gt[:], tab_d[:], idx_t[:], CPC, CPC, OUTF,
                                         single_packet=False)
                    oh3 = ohp.tile([128, TPC, 128], BF16)
                    nc.vector.tensor_tensor(
                        out=oh3[:],
                        in0=iota_t[:].rearrange("p (o f) -> p o f", o=1)
                            .to_broadcast([128, TPC, 128]),
                        in1=dst_t[:].to_broadcast([128, TPC, 128]),
                        op=mybir.AluOpType.is_equal)
                    for t in range(TPC):
                        g = ch * TPC + t
                        col = int(tile_col[g])
                        oh = oh3[:, t, :]
                        if col < 0:
                            sc = pssc.tile([128, 128], F32, space="PSUM", tag="sc")
                            nc.tensor.matmul(out=sc[:], lhsT=oh, rhs=gt[:, t, :],
                                             start=True, stop=True)
                            continue
                        u_in_col = g - int(np.sum(t_col[:col]))
                        grp, ucol = col // 4, col % 4
                        first = (ucol == 0 and u_in_col == 0)
                        last = (ucol == 3 and u_in_col == int(t_col[col]) - 1)
                        if first:
                            ps_g = psp.tile([128, 512], F32, space="PSUM", tag="ps")
                        nc.tensor.matmul(
                            out=ps_g[:, ucol * 128:(ucol + 1) * 128],
                            lhsT=oh, rhs=gt[:, t, :],
                            start=first, stop=last)
                        if last:
                            nc.vector.tensor_copy(
                                out=PART[:, grp * 512:(grp + 1) * 512],
                                in_=ps_g[:])

            # ---------------- exchange: XOR recursive-halving RS ------------
            tc.strict_bb_all_engine_barrier()
            if os.environ.get("SKIP_EXCHANGE"):
                pass
            elif True:
              with tc.tile_critical():
                with nc.named_scope("exchange"):
                    nc.gpsimd.load_library(library_config.remote_dma)
                    nc.gpsimd.bir_kernel_barrier_wait([list(range(P))])
                    stages = [
                        # (xor_k, send_col_lo, ncols, slots, decl)
                        (4, 160, 160, [4, 5, 6, 7], 6),
                        (2, 80, 80, [0, 1, 2, 3], 2),
                        (1, 40, 40, [0, 1, 2, 3], 1),
                    ]
                    npr = 0      # preps issued
                    for s, (k, lo, ncols, slots, decl) in enumerate(stages):
                        sub = ncols // 4
                        for i, slot in enumerate(slots):
                            rdests = [None] * 8
                            rdests[slot] = (0, decl)
                            nc.gpsimd.remote_dma_broadcast(
                                out_ap=RECV[:, (lo - ncols + i * sub) * 128:
                                            (lo - ncols + (i + 1) * sub) * 128],
                                in_ap=PART[:, (lo + i * sub) * 128:
                                           (lo + (i + 1) * sub) * 128],
                                remote_sem=rsem, local_sem=lsem,
                                rdests=rdests).then_inc(psem, 1)
                        npr += 4
                        nc.gpsimd.wait_ge(psem, npr)
                        nc.gpsimd.trigger_dma(count=4)
                        # wait for peer data, then reduce into kept half
                        nc.vector.wait_ge(rsem, 8 * (s + 1))
                        keep = lo - ncols
                        nc.vector.tensor_tensor(
                            out=PART[:, keep * 128:lo * 128],
                            in0=PART[:, keep * 128:lo * 128],
                            in1=RECV[:, keep * 128:lo * 128],
                            op=mybir.AluOpType.add).then_inc(xsem, 1)
                        if s < 2:
                            # notify next-stage peer that our recv buf is free
                            k2 = stages[s + 1][0]
                            nc.gpsimd.wait_ge(xsem, s + 1)
                            rdests = [None] * 8
                            rdests[0] = (0, k2)
                            nc.gpsimd.remote_sem_update_broadcast(
                                remote_sem=nsem, local_sem=lsem,
                                rdests=rdests).then_inc(psem, 1)
                            npr += 1
                            nc.gpsimd.wait_ge(psem, npr)
                            nc.gpsimd.trigger_dma(count=1)
                            nc.gpsimd.wait_ge(nsem, 2 * (s + 1))
                    nc.gpsimd.wait_ge(xsem, 3)
                    nc.gpsimd.wait_ge(lsem, 16 * npr)

            # ---------------- phase 3: own-dst projection + bias + add ------
            if os.environ.get("SKIP_P3"):
                ost0 = stagep.tile([128, 512], F32, tag="ost")
                nc.vector.tensor_copy(out=ost0[:], in_=PART[:, 0:512])
                nc.sync.dma_start(out_d[0:512, :].rearrange("(u p) f -> p u f", p=128),
                                  ost0[:].rearrange("p (u f) -> p u f", u=4))
            elif True:
             with nc.named_scope("phase3"):
                for grp in range(10):
                    b0 = xc2p.tile([128, 512], F32, tag="b0")
                    b1t = xc2p.tile([128, 512], F32, tag="b1")
                    b2t = xc2p.tile([2, 512], F32, tag="b2")
                    nc.sync.dma_start(b0[:], xdT_d[0:128, grp * 512:(grp + 1) * 512])
                    nc.sync.dma_start(b1t[:], xdT_d[128:256, grp * 512:(grp + 1) * 512])
                    nc.sync.dma_start(b2t[:], xdT_d[256:258, grp * 512:(grp + 1) * 512])
                    ps3 = psp.tile([128, 512], F32, space="PSUM", tag="ps")
                    for u in range(4):
                        sl = slice(u * 128, (u + 1) * 128)
                        nc.tensor.matmul(out=ps3[:, sl], lhsT=b0[:, sl],
                                         rhs=w2[:, 0, :], start=(u == 0), stop=False)
                        nc.tensor.matmul(out=ps3[:, sl], lhsT=b1t[:, sl],
                                         rhs=w2[:, 1, :], start=False, stop=False)
                        nc.tensor.matmul(out=ps3[:, sl], lhsT=b2t[:, sl],
                                         rhs=wb[:], start=False, stop=(u == 3))
                    ost = stagep.tile([128, 512], F32, tag="ost")
                    nc.vector.tensor_tensor(
                        out=ost[:], in0=ps3[:],
                        in1=PART[:, grp * 512:(grp + 1) * 512],
                        op=mybir.AluOpType.add)
                    nc.sync.dma_start(
                        out_d[grp * 512:(grp + 1) * 512, :].rearrange(
                            "(u p) f -> p u f", p=128),
                        ost[:].rearrange("p (u f) -> p u f", u=4))

    nc.compile()
    return nc


# ============================ public entry ================================

def _install_ntff_hook():
    """The agent image lacks antenv.axon_hooks; recreate it and register the
    ctypes NTFF profile hook so trace=True works under axon."""
    import types
    import antenv
    if "antenv.axon_hooks" not in sys.modules:
        m = types.ModuleType("antenv.axon_hooks")
        _h = [None]
        m.get_axon_ntff_profile_hook = lambda: _h[0]
        m.set_axon_ntff_profile_hook = lambda h: _h.__setitem__(0, h)
        sys.modules["antenv.axon_hooks"] = m
        antenv.axon_hooks = m
    import antenv.axon_hooks as ah
    if ah.get_axon_ntff_profile_hook() is None:
        try:
            from trn_agent_boot.trn_boot import _ntff_profile_via_ctypes
            ah.set_axon_ntff_profile_hook(
                _ntff_profile_via_ctypes("/opt/axon/libaxon_pjrt.so"))
        except Exception as e:
            print(f"ntff hook install failed ({e}); timing disabled")



def kernel(x, W1, b1, W2, b2, src_ids, dst_ids, n_dst):
    n_dst = int(n_dst)
    assert n_dst == N_DST
    in_maps, sched = _host_prep(x, W1, b1, W2, b2, src_ids, dst_ids)
    key = (sched["nchunk"], tuple(sched["t_col"].tolist()))
    if key not in _CACHE:
        _CACHE.clear()
        _CACHE[key] = _build(sched)
    nc = _CACHE[key]
    trace = bool(os.environ.get("BASS_KERNEL_TRACE"))
    kw = {}
    if trace:
        _install_ntff_hook()
        kw = dict(trace=True, trace_cores=[0], stitch_traces=False)
    res = run_bass_kernel_spmd(nc, in_maps, core_ids=list(range(P)), **kw)
    if trace:
        print(f"HW exec time: {res.exec_time_ns} ns")
        if res.per_core_scope_times:
            for scope, m in sorted(res.per_core_scope_times.items()):
                print(f"  scope {scope}: {m}")
        if res.instructions_and_trace:
            print(f"  trace: {res.instructions_and_trace[1]}")
    out = np.concatenate([res.results[c]["out"] for c in range(P)], axis=0)
    return np.ascontiguousarray(out[:N_DST]).astype(np.float32)


if __name__ == "__main__":
    # smoke test with random data
    rng = np.random.default_rng(0)
    x = rng.standard_normal((N_SRC, INF), dtype=np.float32)
    W1 = rng.standard_normal((OUTF, INF), dtype=np.float32) * 0.0625
    W2 = rng.standard_normal((OUTF, INF), dtype=np.float32) * 0.0625
    b1 = np.zeros(OUTF, np.float32)
    b2 = np.zeros(OUTF, np.float32)
    src = rng.integers(0, N_SRC, N_EDGES).astype(np.int32)
    dst = np.sort(rng.integers(0, N_DST, N_EDGES).astype(np.int32))
    got = kernel(x, W1, b1, W2, b2, src, dst, N_DST)
    proj = x @ W1.T + b1
    want = np.zeros((N_DST, OUTF), np.float32)
    np.add.at(want, dst, proj[src])
    want += x[:N_DST] @ W2.T + b2
    denom = np.abs(want).max()
    print("rel err:", np.abs(got - want).max() / denom)


# revision 9
# speedup vs baseline: 1.4467x; 1.1969x over previous
"""DistSageConv on 8 TRN2 NeuronCores (Bass/Tile).

Reference computation:
    out  = x @ W1.T + b1                                  # [n_src, 128]
    out1 = segment_sum(out[src_ids], dst_ids, n_dst)      # [n_dst, 128]
    out5 = x[:n_dst] @ W2.T + b2
    return out5 + out1

Distribution (the module's own design): src nodes sharded across 8 cores;
each core projects its own src shard into a local bf16 table, gathers its
own-src edges' rows with SWDGE dma_gather, segment-reduces them with one-hot
matmuls on the PE (dst grouped into 128-row blocks), then the per-core
partial aggregates [40960, 128] are reduce-scattered to block owners with a
3-stage XOR recursive-halving exchange over SBUF remote DMA
(remote_dma_broadcast relative dests). Owners fuse x[:n_dst] @ W2.T (+bias,
+deg*b1 via an augmented K=258 matmul) and write their 5120-row output slab.

The per-core partial slab is stored in an XOR-permuted owner-major column
order (column j on core c holds block (((j//40) ^ c) * 40 + j % 40)), which
makes the send/recv/add slabs of every exchange stage the same static column
ranges on all cores: a single SPMD program with no data-dependent control.
"""
import sys
sys.path.insert(0, "/opt/trn_rl_repo")

import numpy as np
import ml_dtypes

import os
import concourse.bacc as bacc
import concourse.bass as bass
import concourse.mybir as mybir
import concourse.tile as tile
from concourse import library_config
from concourse.bass_utils import run_bass_kernel_spmd

# ---------------- problem constants (hardcoded per contract) --------------
P = 8                      # cores
N_SRC = 100000
N_DST = 40000
N_EDGES = 640000
INF = 256                  # in_feats
OUTF = 128                 # out_feats
SRC_SH = N_SRC // P        # 12500 src rows per core
SRC_PAD = 12800            # padded table rows (25 x 512)
NBLK = 320                 # padded dst blocks of 128 (40960 dst rows)
BPC = NBLK // P            # 40 blocks (columns) owned per core
DST_PAD = NBLK * 128       # 40960
CPC = 4096                 # gather chunk edges
TPC = CPC // 128           # 64 tiles per chunk

F32 = mybir.dt.float32
BF16 = mybir.dt.bfloat16
I16 = mybir.dt.int16

_CACHE = {}


# ============================ host-side prep ==============================

def _wrap_idxs(idx):
    """[n] int16 -> [128, n//16] wrapped in 16 partitions, replicated x8."""
    n = len(idx)
    w = np.zeros((128, n // 16), dtype=np.int16)
    for p in range(16):
        w[p, :] = idx[p::16]
    for r in range(1, 8):
        w[16 * r:16 * r + 16, :] = w[:16, :]
    return w


def _host_prep(x, W1, b1, W2, b2, src_ids, dst_ids):
    """Build per-core input arrays + the static tile->column schedule."""
    x = np.asarray(x, np.float32)
    W1 = np.asarray(W1, np.float32)
    W2 = np.asarray(W2, np.float32)
    b1 = np.asarray(b1, np.float32).reshape(-1)
    b2 = np.asarray(b2, np.float32).reshape(-1)
    src_ids = np.asarray(src_ids, np.int64)
    dst_ids = np.asarray(dst_ids, np.int64)

    owner = src_ids // SRC_SH                       # edge -> src-owner core
    blk = dst_ids // 128                            # edge -> dst block
    deg_full = np.bincount(dst_ids, minlength=DST_PAD).astype(np.float32)

    # per-(core, column) counts;  column j on core c holds block beta_c(j)
    per_core = []
    for c in range(P):
        m = owner == c
        e_src = (src_ids[m] - c * SRC_SH).astype(np.int64)
        e_dst = dst_ids[m]
        e_blk = blk[m]
        col = ((e_blk // BPC) ^ c) * BPC + (e_blk % BPC)
        order = np.argsort(col, kind="stable")
        per_core.append((e_src[order], e_dst[order], col[order]))

    counts = np.zeros((P, NBLK), dtype=np.int64)
    for c in range(P):
        counts[c] = np.bincount(per_core[c][2], minlength=NBLK)
    t_col = np.maximum(1, (counts.max(axis=0) + 127) // 128)   # tiles per col
    nt_tot = int(t_col.sum())
    nt_pad = ((nt_tot + TPC - 1) // TPC) * TPC
    nchunk = nt_pad // TPC
    col_base = np.zeros(NBLK + 1, dtype=np.int64)
    col_base[1:] = np.cumsum(t_col)

    # static schedule: per tile -> (col, start, stop) ;  col -1 => pad tile
    tile_col = np.full(nt_pad, -1, dtype=np.int64)
    for j in range(NBLK):
        tile_col[col_base[j]:col_base[j + 1]] = j

    in_maps = []
    iota = np.broadcast_to(np.arange(128, dtype=np.float32), (128, 128))
    iota = np.ascontiguousarray(iota.astype(ml_dtypes.bfloat16))
    W1T = np.ascontiguousarray(W1.T)                          # [256, 128]
    W2T_aug = np.concatenate([W2.T, b1[None, :], b2[None, :]], axis=0)
    W2T_aug = np.ascontiguousarray(W2T_aug.astype(np.float32))  # [258, 128]

    for c in range(P):
        e_src, e_dst, e_col = per_core[c]
        # scatter edges into padded per-column tile runs
        idx_arr = np.zeros(nt_pad * 128, dtype=np.int16)
        dst_arr = np.full(nt_pad * 128, -1.0, dtype=np.float32)
        cc = np.bincount(e_col, minlength=NBLK)
        # position of each edge inside its column run
        pos_in_col = np.zeros(len(e_col), dtype=np.int64)
        cstart = np.zeros(NBLK + 1, dtype=np.int64)
        cstart[1:] = np.cumsum(cc)
        pos_in_col = np.arange(len(e_col)) - cstart[e_col]
        gpos = col_base[e_col] * 128 + pos_in_col
        idx_arr[gpos] = e_src.astype(np.int16)
        e_blk_of_col = ((e_col // BPC) ^ c) * BPC + (e_col % BPC)
        dst_arr[gpos] = (e_dst - e_blk_of_col * 128).astype(np.float32)

        idx_dram = np.zeros((nchunk, 128, CPC // 16), dtype=np.int16)
        for ch in range(nchunk):
            idx_dram[ch] = _wrap_idxs(idx_arr[ch * CPC:(ch + 1) * CPC])
        # dstloc layout per chunk: [128 part(edge-in-tile), TPC]
        dst_dram = dst_arr.reshape(nchunk, TPC, 128).transpose(0, 2, 1)
        dst_dram = np.ascontiguousarray(dst_dram.astype(ml_dtypes.bfloat16))

        xT = np.zeros((INF, SRC_PAD), dtype=np.float32)
        xT[:, :SRC_SH] = x[c * SRC_SH:(c + 1) * SRC_SH].T
        lo, hi = c * 5120, min((c + 1) * 5120, N_DST)
        xdT = np.zeros((INF + 2, 5120), dtype=np.float32)
        if hi > lo:
            xdT[:INF, :hi - lo] = x[lo:hi].T
        xdT[INF, :] = deg_full[c * 5120:(c + 1) * 5120]
        xdT[INF + 1, :] = 1.0

        in_maps.append({
            "xT": xT,
            "xdT": np.ascontiguousarray(xdT),
            "W1T": W1T,
            "W2Ta": W2T_aug,
            "iota": iota,
            "idx": idx_dram,
            "dstloc": dst_dram,
        })

    sched = {"t_col": t_col, "nchunk": nchunk, "tile_col": tile_col}
    return in_maps, sched


# ============================ device program ==============================

def _build(sched):
    t_col = sched["t_col"]
    nchunk = sched["nchunk"]
    tile_col = sched["tile_col"]
    nt_pad = nchunk * TPC
    COLW = 128 * 2            # bf16 bytes per column per partition (unused)

    nc = bacc.Bacc("TRN2", target_bir_lowering=False, debug=False,
                   num_devices=P)

    xT_d = nc.dram_tensor("xT", [INF, SRC_PAD], F32, kind="ExternalInput")
    xdT_d = nc.dram_tensor("xdT", [INF + 2, 5120], F32, kind="ExternalInput")
    W1T_d = nc.dram_tensor("W1T", [INF, OUTF], F32, kind="ExternalInput")
    W2Ta_d = nc.dram_tensor("W2Ta", [INF + 2, OUTF], F32, kind="ExternalInput")
    iota_d = nc.dram_tensor("iota", [128, 128], BF16, kind="ExternalInput")
    idx_d = nc.dram_tensor("idx", [nchunk, 128, CPC // 16], I16, kind="ExternalInput")
    dst_d = nc.dram_tensor("dstloc", [nchunk, 128, TPC], BF16, kind="ExternalInput")
    out_d = nc.dram_tensor("out", [5120, OUTF], F32, kind="ExternalOutput")
    tab_d = nc.dram_tensor("tab", [SRC_PAD, OUTF], BF16, kind="Internal")

    # persistent SBUF: partial slab + exchange recv
    PART = nc.alloc_sbuf_tensor("part", [128, NBLK * 128], BF16)
    RECV = nc.alloc_sbuf_tensor("recv", [128, (NBLK // 2) * 128], BF16)

    rsem = nc.semaphore("rsem").__enter__()      # data arrived (remote inc)
    nsem = nc.semaphore("nsem").__enter__()      # notify: peer consumed recv
    lsem = nc.semaphore("lsem").__enter__()      # local send drained
    psem = nc.semaphore("psem").__enter__()      # desc-gen complete
    xsem = nc.semaphore("xsem").__enter__()      # DVE add done

    with tile.TileContext(nc) as tc:
        nc.gpsimd.load_library(library_config.mlp)
        with (
            tc.tile_pool(name="consts", bufs=1) as constp,
            tc.tile_pool(name="xab", bufs=3) as xabp,
            tc.tile_pool(name="xc2", bufs=2) as xc2p,
            tc.tile_pool(name="stage", bufs=3) as stagep,
            tc.tile_pool(name="idx", bufs=4) as idxp,
            tc.tile_pool(name="dstl", bufs=4) as dstlp,
            tc.tile_pool(name="gath", bufs=3) as gathp,
            tc.tile_pool(name="oh", bufs=3) as ohp,
            tc.tile_pool(name="ps", bufs=2, space="PSUM") as psp,
            tc.tile_pool(name="pssc", bufs=1, space="PSUM") as pssc,
        ):
            # ---- constants
            iota_t = constp.tile([128, 128], BF16)
            nc.sync.dma_start(iota_t[:], iota_d[:])
            w1 = constp.tile([128, 2, OUTF], F32)
            nc.sync.dma_start(w1[:], W1T_d[:].rearrange("(k p) f -> p k f", p=128))
            w2 = constp.tile([128, 2, OUTF], F32)
            nc.sync.dma_start(w2[:], W2Ta_d[:INF].rearrange("(k p) f -> p k f", p=128))
            wb = constp.tile([2, OUTF], F32)
            nc.sync.dma_start(wb[:], W2Ta_d[INF:INF + 2, :])

            # ---------------- phase 1: project own src shard ----------------
            with nc.named_scope("phase1"):
                for j in range(SRC_PAD // 512):
                    a0 = xabp.tile([128, 512], F32, tag="a0")
                    a1 = xabp.tile([128, 512], F32, tag="a1")
                    nc.sync.dma_start(a0[:], xT_d[0:128, j * 512:(j + 1) * 512])
                    nc.sync.dma_start(a1[:], xT_d[128:256, j * 512:(j + 1) * 512])
                    ps = psp.tile([128, 512], F32, space="PSUM", tag="ps")
                    for u in range(4):
                        nc.tensor.matmul(
                            out=ps[:, u * 128:(u + 1) * 128],
                            lhsT=a0[:, u * 128:(u + 1) * 128], rhs=w1[:, 0, :],
                            start=(u == 0), stop=False)
                        nc.tensor.matmul(
                            out=ps[:, u * 128:(u + 1) * 128],
                            lhsT=a1[:, u * 128:(u + 1) * 128], rhs=w1[:, 1, :],
                            start=False, stop=(u == 3))
                    st = stagep.tile([128, 512], BF16, tag="st1")
                    nc.vector.tensor_copy(out=st[:], in_=ps[:])
                    nc.sync.dma_start(
                        tab_d[j * 512:(j + 1) * 512, :].rearrange(
                            "(u p) f -> p u f", p=128),
                        st[:].rearrange("p (u f) -> p u f", u=4))

            # ---------------- phase 2: gather + segment matmul --------------
            if os.environ.get("SKIP_P2"):
                pass
            elif True:
             with nc.named_scope("phase2"):
                ps_g = None
                for ch in range(nchunk):
                    idx_t = idxp.tile([128, CPC // 16], I16)
                    nc.sync.dma_start(idx_t[:], idx_d[ch])
                    dst_t = dstlp.tile([128, TPC], BF16)
                    nc.sync.dma_start(dst_t[:], dst_d[ch])
                    gt = gathp.tile([128, TPC, 128], BF16)
                    nc.gpsimd.dma_gather(gt[:], tab_d[:], idx_t[:], CPC, CPC, OUTF,
                                         single_packet=False)
                    oh3 = ohp.tile([128, TPC, 128], BF16)
                    nc.vector.tensor_tensor(
                        out=oh3[:],
                        in0=iota_t[:].rearrange("p (o f) -> p o f", o=1)
                            .to_broadcast([128, TPC, 128]),
                        in1=dst_t[:].to_broadcast([128, TPC, 128]),
                        op=mybir.AluOpType.is_equal)
                    for t in range(TPC):
                        g = ch * TPC + t
                        col = int(tile_col[g])
                        oh = oh3[:, t, :]
                        if col < 0:
                            sc = pssc.tile([128, 128], F32, space="PSUM", tag="sc")
                            nc.tensor.matmul(out=sc[:], lhsT=oh, rhs=gt[:, t, :],
                                             start=True, stop=True)
                            continue
                        u_in_col = g - int(np.sum(t_col[:col]))
                        grp, ucol = col // 4, col % 4
                        first = (ucol == 0 and u_in_col == 0)
                        last = (ucol == 3 and u_in_col == int(t_col[col]) - 1)
                        if first:
                            ps_g = psp.tile([128, 512], F32, space="PSUM", tag="ps")
                        nc.tensor.matmul(
                            out=ps_g[:, ucol * 128:(ucol + 1) * 128],
                            lhsT=oh, rhs=gt[:, t, :],
                            start=first, stop=last)
                        if last:
                            nc.vector.tensor_copy(
                                out=PART[:, grp * 512:(grp + 1) * 512],
                                in_=ps_g[:])

            # ---------------- exchange: XOR recursive-halving RS ------------
            tc.strict_bb_all_engine_barrier()
            if os.environ.get("SKIP_EXCHANGE"):
                pass
            elif True:
              with tc.tile_critical():
                with nc.named_scope("exchange"):
                    nc.gpsimd.load_library(library_config.remote_dma)
                    nc.gpsimd.bir_kernel_barrier_wait([list(range(P))])
                    stages = [
                        # (xor_k, send_col_lo, ncols, slots, decl)
                        (4, 160, 160, [4, 5, 6, 7], 6),
                        (2, 80, 80, [0, 1, 2, 3], 2),
                        (1, 40, 40, [0, 1, 2, 3], 1),
                    ]
                    npr = 0      # preps issued
                    for s, (k, lo, ncols, slots, decl) in enumerate(stages):
                        sub = ncols // 4
                        for i, slot in enumerate(slots):
                            rdests = [None] * 8
                            rdests[slot] = (0, decl)
                            nc.gpsimd.remote_dma_broadcast(
                                out_ap=RECV[:, (lo - ncols + i * sub) * 128:
                                            (lo - ncols + (i + 1) * sub) * 128],
                                in_ap=PART[:, (lo + i * sub) * 128:
                                           (lo + (i + 1) * sub) * 128],
                                remote_sem=rsem, local_sem=lsem,
                                rdests=rdests).then_inc(psem, 1)
                        npr += 4
                        nc.gpsimd.wait_ge(psem, npr)
                        nc.gpsimd.trigger_dma(count=4)
                        # wait for peer data, then reduce into kept half
                        nc.vector.wait_ge(rsem, 8 * (s + 1))
                        keep = lo - ncols
                        nc.vector.tensor_tensor(
                            out=PART[:, keep * 128:lo * 128],
                            in0=PART[:, keep * 128:lo * 128],
                            in1=RECV[:, keep * 128:lo * 128],
                            op=mybir.AluOpType.add).then_inc(xsem, 1)
                        if s < 2:
                            # notify next-stage peer that our recv buf is free
                            k2 = stages[s + 1][0]
                            nc.gpsimd.wait_ge(xsem, s + 1)
                            rdests = [None] * 8
                            rdests[0] = (0, k2)
                            nc.gpsimd.remote_sem_update_broadcast(
                                remote_sem=nsem, local_sem=lsem,
                                rdests=rdests).then_inc(psem, 1)
                            npr += 1
                            nc.gpsimd.wait_ge(psem, npr)
                            nc.gpsimd.trigger_dma(count=1)
                            nc.gpsimd.wait_ge(nsem, 2 * (s + 1))
                    nc.gpsimd.wait_ge(xsem, 3)
                    nc.gpsimd.wait_ge(lsem, 16 * npr)

            # ---------------- phase 3: own-dst projection + bias + add ------
            if os.environ.get("SKIP_P3"):
                ost0 = stagep.tile([128, 512], F32, tag="ost")
                nc.vector.tensor_copy(out=ost0[:], in_=PART[:, 0:512])
                nc.sync.dma_start(out_d[0:512, :].rearrange("(u p) f -> p u f", p=128),
                                  ost0[:].rearrange("p (u f) -> p u f", u=4))
            elif True:
             with nc.named_scope("phase3"):
                for grp in range(10):
                    b0 = xc2p.tile([128, 512], F32, tag="b0")
                    b1t = xc2p.tile([128, 512], F32, tag="b1")
                    b2t = xc2p.tile([2, 512], F32, tag="b2")
                    nc.sync.dma_start(b0[:], xdT_d[0:128, grp * 512:(grp + 1) * 512])
                    nc.sync.dma_start(b1t[:], xdT_d[128:256, grp * 512:(grp + 1) * 512])
                    nc.sync.dma_start(b2t[:], xdT_d[256:258, grp * 512:(grp + 1) * 512])
                    ps3 = psp.tile([128, 512], F32, space="PSUM", tag="ps")
                    for u in range(4):
                        sl = slice(u * 128, (u + 1) * 128)
                        nc.tensor.matmul(out=ps3[:, sl], lhsT=b0[:, sl],
                                         rhs=w2[:, 0, :], start=(u == 0), stop=False)
                        nc.tensor.matmul(out=ps3[:, sl], lhsT=b1t[:, sl],
                                         rhs=w2[:, 1, :], start=False, stop=False)
                        nc.tensor.matmul(out=ps3[:, sl], lhsT=b2t[:, sl],
                                         rhs=wb[:], start=False, stop=(u == 3))
                    ost = stagep.tile([128, 512], F32, tag="ost")
                    nc.vector.tensor_tensor(
                        out=ost[:], in0=ps3[:],
                        in1=PART[:, grp * 512:(grp + 1) * 512],
                        op=mybir.AluOpType.add)
                    nc.sync.dma_start(
                        out_d[grp * 512:(grp + 1) * 512, :].rearrange(
                            "(u p) f -> p u f", p=128),
                        ost[:].rearrange("p (u f) -> p u f", u=4))

    nc.compile()
    return nc


# ============================ public entry ================================

def _install_ntff_hook():
    """The agent image lacks antenv.axon_hooks; recreate it and register the
    ctypes NTFF profile hook so trace=True works under axon."""
    import types
    import antenv
    if "antenv.axon_hooks" not in sys.modules:
        m = types.ModuleType("antenv.axon_hooks")
        _h = [None]
        m.get_axon_ntff_profile_hook = lambda: _h[0]
        m.set_axon_ntff_profile_hook = lambda h: _h.__setitem__(0, h)
        sys.modules["antenv.axon_hooks"] = m
        antenv.axon_hooks = m
    import antenv.axon_hooks as ah
    if ah.get_axon_ntff_profile_hook() is None:
        try:
            from trn_agent_boot.trn_boot import _ntff_profile_via_ctypes
            ah.set_axon_ntff_profile_hook(
                _ntff_profile_via_ctypes("/opt/axon/libaxon_pjrt.so"))
        except Exception as e:
            print(f"ntff hook install failed ({e}); timing disabled")



def kernel(x, W1, b1, W2, b2, src_ids, dst_ids, n_dst):
    n_dst = int(n_dst)
    assert n_dst == N_DST
    in_maps, sched = _host_prep(x, W1, b1, W2, b2, src_ids, dst_ids)
    key = (sched["nchunk"], tuple(sched["t_col"].tolist()))
    if key not in _CACHE:
        _CACHE.clear()
        _CACHE[key] = _build(sched)
    nc = _CACHE[key]
    trace = bool(os.environ.get("BASS_KERNEL_TRACE"))
    kw = {}
    if trace:
        _install_ntff_hook()
        kw = dict(trace=True, trace_cores=[0], stitch_traces=False)
    res = run_bass_kernel_spmd(nc, in_maps, core_ids=list(range(P)), **kw)
    if trace:
        print(f"HW exec time: {res.exec_time_ns} ns")
        if res.per_core_scope_times:
            for scope, m in sorted(res.per_core_scope_times.items()):
                print(f"  scope {scope}: {m}")
        if res.instructions_and_trace:
            print(f"  trace: {res.instructions_and_trace[1]}")
    out = np.concatenate([res.results[c]["out"] for c in range(P)], axis=0)
    return np.ascontiguousarray(out[:N_DST]).astype(np.float32)


if __name__ == "__main__":
    # smoke test with random data
    rng = np.random.default_rng(0)
    x = rng.standard_normal((N_SRC, INF), dtype=np.float32)
    W1 = rng.standard_normal((OUTF, INF), dtype=np.float32) * 0.0625
    W2 = rng.standard_normal((OUTF, INF), dtype=np.float32) * 0.0625
    b1 = np.zeros(OUTF, np.float32)
    b2 = np.zeros(OUTF, np.float32)
    src = rng.integers(0, N_SRC, N_EDGES).astype(np.int32)
    dst = np.sort(rng.integers(0, N_DST, N_EDGES).astype(np.int32))
    got = kernel(x, W1, b1, W2, b2, src, dst, N_DST)
    proj = x @ W1.T + b1
    want = np.zeros((N_DST, OUTF), np.float32)
    np.add.at(want, dst, proj[src])
    want += x[:N_DST] @ W2.T + b2
    denom = np.abs(want).max()
    print("rel err:", np.abs(got - want).max() / denom)
